# revision 43
# baseline (speedup 1.0000x reference)
"""Trainium2 Bass kernel for nn_Attention_43868795961547 (sparse_attention).

Reference computation per batch item (8 items, data-parallel over 8 cores):
  x  = LN(img[b]) @ w_qkv -> q,k,v (8 heads x 64)          [1024 tokens]
  kt,vt from LN(tab[b]) @ w_tab_qkv appended as key/value position 1024
  out = softmax(q k^T / 8) @ v ; out @ w_out + b_out        -> [1024, 512]

v4 strategy (per core), all matmuls bf16 (216ns/MM streaming floor at N=512):
  - LN: bn_stats fp32, rstd via one ACT Rsqrt, xn bf16, PE transpose into a
    bf16 PSUM view; ln affine fused into the PSUM->SBUF move (split ACT/DVE).
  - qT,kT feature-major bf16 (PSUM copies split DVE/ACT per q half);
    v token-major with per-head interleaved ones column (65-col groups) so
    attn@v emits softmax denominators for free.
  - tab token: k_t dots for all 8 heads in ONE packed M=8 accumulated matmul,
    exp'd once; tab's rank-1 output term enters the final projection as a
    K=8 accumulation row (lhsT = normalized tab weights na8, rhs = v_t@w_out).
  - dots^T[kpos, q] K=64 head pairs row-packed at array rows 0/64 (overlap);
    exp on ACT from PSUM (scale folded), ae bf16.
  - attn@v interleaved with the next pair's dots at kpos-chunk granularity so
    the PE never stalls long on exp drains; po0/po1 both live (2 PSUM banks).
  - denominators DVE-moved to a 32-strided staging layout, normalized per
    head pair as soon as its attn@v lands: recip_approx_fast + DMA-broadcast
    (DRAM bounce); the last two pairs use a PE ones-matmul broadcast instead
    to keep the kernel tail off the DMA latency chain.
"""

import numpy as np

import concourse.bass as bass
import concourse.mybir as mybir
import concourse.tile as tile
from concourse import bacc
from concourse import bass_utils
from concourse.masks import make_identity

F32 = mybir.dt.float32
BF16 = mybir.dt.bfloat16

N_CORES = 8
NTOK = 1024  # img tokens per batch item
DIM = 512
HEADS = 8
DHEAD = 64
INNER = 512
SCALE = DHEAD ** -0.5  # 0.125
EPS = 1e-5

NT = NTOK // 128   # 8 token tiles
NC_ = DIM // 128   # 4 feature chunks
NQB = 2            # q blocks of 512
QB = 512


def build_program():
    nc = bacc.Bacc(
        "TRN2",
        target_bir_lowering=False,
        debug=False,
        enable_asserts=False,
        num_devices=N_CORES,
    )

    img = nc.dram_tensor("img_s", [NTOK, DIM], F32, kind="ExternalInput").ap()
    tab = nc.dram_tensor("tab_s", [1, DIM], F32, kind="ExternalInput").ap()
    w_qkv = nc.dram_tensor("wq_b", [DIM, 3 * INNER], BF16, kind="ExternalInput").ap()
    # only k,v columns of w_tab_qkv (cols 512:1536) are used
    w_tab = nc.dram_tensor("wt_b", [DIM, 2 * INNER], BF16, kind="ExternalInput").ap()
    w_out = nc.dram_tensor("wo_b", [INNER, DIM], BF16, kind="ExternalInput").ap()
    b_out = nc.dram_tensor("b_out", [1, DIM], F32, kind="ExternalInput").ap()
    ln_w = nc.dram_tensor("ln_w", [1, DIM], F32, kind="ExternalInput").ap()
    ln_b = nc.dram_tensor("ln_b", [1, DIM], F32, kind="ExternalInput").ap()
    out_d = nc.dram_tensor("out_s", [NTOK, DIM], F32, kind="ExternalOutput").ap()
    dscr = nc.dram_tensor("dscratch", [HEADS, NTOK], BF16, kind="Internal").ap()
    import os
    dbg = None
    if os.environ.get("KERNEL_DEBUG_DUMP"):
        dbg = {
            "dbg_qT0": nc.dram_tensor("dbg_qT0", [128, NTOK], BF16, kind="ExternalOutput").ap(),
            "dbg_kT0": nc.dram_tensor("dbg_kT0", [128, NTOK], BF16, kind="ExternalOutput").ap(),
            "dbg_v0": nc.dram_tensor("dbg_v0", [128, 520], BF16, kind="ExternalOutput").ap(),
            "dbg_dall": nc.dram_tensor("dbg_dall", [128, 2 * NTOK], F32, kind="ExternalOutput").ap(),
            "dbg_rcp": nc.dram_tensor("dbg_rcp", [128, 2 * NTOK], F32, kind="ExternalOutput").ap(),
            "dbg_na8": nc.dram_tensor("dbg_na8", [8, NTOK], BF16, kind="ExternalOutput").ap(),
            "dbg_te8": nc.dram_tensor("dbg_te8", [8, NTOK], F32, kind="ExternalOutput").ap(),
            "dbg_ouT0": nc.dram_tensor("dbg_ouT0", [128, NTOK], BF16, kind="ExternalOutput").ap(),
            "dbg_ouT3": nc.dram_tensor("dbg_ouT3", [128, NTOK], BF16, kind="ExternalOutput").ap(),
        }

    with tile.TileContext(nc) as tc:
        kernel_body(tc, img, tab, w_qkv, w_tab, w_out, b_out, ln_w, ln_b, out_d,
                    dscr, dbg)

    nc.compile()
    return nc


def kernel_body(tc, img, tab, w_qkv, w_tab, w_out, b_out, ln_w, ln_b, out_d,
                dscr, dbg=None):
    nc = tc.nc
    AF = mybir.ActivationFunctionType
    OP = mybir.AluOpType

    import contextlib
    ctx = contextlib.ExitStack()
    with ctx:
        # ---------------- pools ----------------
        const_p = ctx.enter_context(tc.tile_pool(name="const", bufs=1))
        qkT_p = ctx.enter_context(tc.tile_pool(name="qkT", bufs=1))
        v_p = ctx.enter_context(tc.tile_pool(name="vp", bufs=1))
        outuT_p = ctx.enter_context(tc.tile_pool(name="outuT", bufs=1))
        small_p = ctx.enter_context(tc.tile_pool(name="smallp", bufs=1))
        w_p = ctx.enter_context(tc.tile_pool(name="wp", bufs=1))
        ln_p = ctx.enter_context(tc.tile_pool(name="lnp", bufs=2))
        xnT_p = ctx.enter_context(tc.tile_pool(name="xnt", bufs=1))
        # bufs=3: an av_dots block writes the NEXT pair's ae while this
        # pair's ae is still being read — reuse must skip one block
        ae_p = ctx.enter_context(tc.tile_pool(name="aep", bufs=3))
        bc_p = ctx.enter_context(tc.tile_pool(name="bcp", bufs=2))
        fo_p = ctx.enter_context(tc.tile_pool(name="fout", bufs=2))

        # psum (8 banks): big 2 tags x [128,1024]f32 (4) + po 2 bufs [128,512]
        # (2) + fin 2 bufs [128,512] (2)
        psum_big = ctx.enter_context(tc.tile_pool(name="psbig", bufs=1, space="PSUM"))
        psum_o = ctx.enter_context(tc.tile_pool(name="pso", bufs=2, space="PSUM"))
        psum_f = ctx.enter_context(tc.tile_pool(name="psf", bufs=2, space="PSUM"))
        bigctr = [0]

        def big_tile(name):
            t = psum_big.tile([128, 2 * QB], F32, name=name,
                              tag=f"big{bigctr[0] % 2}")
            bigctr[0] += 1
            return t

        # ---------------- constants (identity first: gates transposes) ----
        identb = const_p.tile([128, 128], BF16, name="identb")
        make_identity(nc, identb)

        # ---------------- input DMA (img on sync first: LN gates the PE) --
        x_ts = []
        for t in range(NT):
            x_t = ln_p.tile([128, DIM], F32, name="x_t", tag="x_t", bufs=8)
            nc.sync.dma_start(out=x_t, in_=img[t * 128:(t + 1) * 128, :])
            x_ts.append(x_t)
        tb = ln_p.tile([1, DIM], F32, name="tb", tag="tb", bufs=1)
        nc.gpsimd.dma_start(out=tb, in_=tab)

        lnw_bc = const_p.tile([1, DIM], F32, name="lnw_bc")
        lnb_bc = const_p.tile([1, DIM], F32, name="lnb_bc")
        bout_bc = const_p.tile([128, DIM], F32, name="bout_bc")
        nc.sync.dma_start(out=lnw_bc, in_=ln_w)
        nc.sync.dma_start(out=lnb_bc, in_=ln_b)

        wq = []
        for c in range(NC_):
            t = w_p.tile([128, 3 * INNER], BF16, name=f"wq{c}", tag=f"wq{c}")
            nc.sync.dma_start(out=t, in_=w_qkv[c * 128:(c + 1) * 128, :])
            wq.append(t)
        wt = []
        for c in range(NC_):
            t = w_p.tile([128, 2 * INNER], BF16, name=f"wt{c}", tag=f"wt{c}")
            nc.gpsimd.dma_start(out=t, in_=w_tab[c * 128:(c + 1) * 128, :])
            wt.append(t)
        wo = []
        for c in range(NC_):
            t = w_p.tile([128, DIM], BF16, name=f"wo{c}", tag=f"wo{c}")
            nc.gpsimd.dma_start(out=t, in_=w_out[c * 128:(c + 1) * 128, :])
            wo.append(t)
        nc.gpsimd.dma_start(out=bout_bc, in_=b_out.to_broadcast([128, DIM]))

        eps_t = const_p.tile([128, 1], F32, name="eps_t")
        nc.vector.memset(eps_t, EPS)
        ones64 = const_p.tile([128, 64], BF16, name="ones64")
        nc.vector.memset(ones64, 1.0)
        identf = const_p.tile([1, 1], F32, name="identf")
        nc.vector.memset(identf, 1.0)

        # ln_w / ln_b as feature-major columns: lnwc[p, c] = ln_w[128c + p]
        lnwc = const_p.tile([128, NC_], F32, name="lnwc")
        lnbc = const_p.tile([128, NC_], F32, name="lnbc")
        for c in range(NC_):
            for colt, bc_src in ((lnwc, lnw_bc), (lnbc, lnb_bc)):
                pcol = psum_f.tile([128, 1], F32, name="pcol", tag="fin")
                nc.tensor.transpose(out=pcol, in_=bc_src[0:1, c * 128:(c + 1) * 128],
                                    identity=identf)
                nc.vector.tensor_copy(out=colt[:, c:c + 1], in_=pcol)

        # ---------------- persistent activations ----------------
        xnT = [xnT_p.tile([128, NTOK], BF16, name=f"xnT{c}", tag=f"xnT{c}")
               for c in range(NC_)]
        qT = [qkT_p.tile([128, NTOK], BF16, name=f"qT{c}", tag=f"qT{c}") for c in range(NC_)]
        kT = [qkT_p.tile([128, NTOK], BF16, name=f"kT{c}", tag=f"kT{c}") for c in range(NC_)]
        # v token-major with interleaved ones column per head: 8 x (64+1) = 520
        v_sb = [v_p.tile([128, 520], BF16, name=f"v{t}", tag=f"v{t}") for t in range(NT)]
        # unnormalized out^T chunks [128, 1024] (bf16)
        outuT = [outuT_p.tile([128, NTOK], BF16, name=f"ouT{c}", tag=f"ouT{c}")
                 for c in range(NC_)]

        # tab small tiles
        tnT = small_p.tile([128, NC_], BF16, name="tnT")      # tab LN^T columns
        k_tT = small_p.tile([128, NC_], BF16, name="k_tT")    # tab key, feat-major
        v_tT = small_p.tile([128, NC_], BF16, name="v_tT")    # tab value, feat-major
        kpad = small_p.tile([128, 32], BF16, name="kpad")     # zero-padded key blocks
        vpad = small_p.tile([128, 32], BF16, name="vpad")     # zero-padded value blocks
        W_vt = small_p.tile([8, DIM], BF16, name="W_vt")      # v_t @ w_out rows
        # packed [h, qb*512] layouts (8 rows)
        tabexp8 = small_p.tile([8, NTOK], F32, name="tabexp8")  # exp(tab dots)
        rcp8 = small_p.tile([8, NTOK], F32, name="rcp8")
        na8 = small_p.tile([8, NTOK], BF16, name="na8")       # normalized tab weights
        # strided staging [32*(h%4), (h//4)*1024 + qb*512] (quadrant-legal
        # targets for DVE moves out of PSUM partition 64)
        tabstr = small_p.tile([128, 2 * NTOK], F32, name="tabstr")
        dall = small_p.tile([128, 2 * NTOK], F32, name="dall")
        rcp_s = small_p.tile([128, 2 * NTOK], F32, name="rcp_s")
        rcpb_s = small_p.tile([128, 2 * NTOK], BF16, name="rcpb_s")

        def slot(h):
            return 32 * (h % 4), (h // 4) * NTOK  # (row, col block base)

        nc.gpsimd.memset(kpad, 0.0)
        nc.gpsimd.memset(vpad, 0.0)

        # ---------------- phase 1: img LN + transpose ----------------
        for t in range(NT):
            x_t = x_ts[t]
            stats = ln_p.tile([128, 6], F32, name="stats", tag="stats")
            nc.vector.bn_stats(out=stats, in_=x_t)
            mv = ln_p.tile([128, 2], F32, name="mv", tag="mv")
            nc.vector.bn_aggr(out=mv, in_=stats)
            sd = ln_p.tile([128, 1], F32, name="sd", tag="sd")
            nc.scalar.activation(out=sd, in_=mv[:, 1:2], func=AF.Sqrt,
                                 bias=eps_t, scale=1.0)
            rstd = ln_p.tile([128, 1], F32, name="rstd", tag="rstd")
            nc.vector.reciprocal(out=rstd, in_=sd)

            xn_t = ln_p.tile([128, DIM], BF16, name="xn_t", tag="xn_t", bufs=4)
            nc.vector.tensor_scalar(out=xn_t, in0=x_t,
                                    scalar1=mv[:, 0:1], scalar2=rstd,
                                    op0=OP.subtract, op1=OP.mult)
            # transpose 4 chunks into one bf16 psum view, ln affine fused
            # into the PSUM->SBUF move (2 chunks on ACT, 2 on DVE)
            ptb = big_tile("pt").bitcast(BF16)  # [128, 2048] bf16 view
            for c in range(NC_):
                nc.tensor.transpose(out=ptb[:, c * 128:(c + 1) * 128],
                                    in_=xn_t[:, c * 128:(c + 1) * 128],
                                    identity=identb)
            for c in range(NC_):
                if c < 2:
                    nc.scalar.activation(
                        out=xnT[c][:, t * 128:(t + 1) * 128],
                        in_=ptb[:, c * 128:(c + 1) * 128],
                        func=AF.Identity,
                        scale=lnwc[:, c:c + 1], bias=lnbc[:, c:c + 1])
                else:
                    nc.vector.tensor_scalar(
                        out=xnT[c][:, t * 128:(t + 1) * 128],
                        in0=ptb[:, c * 128:(c + 1) * 128],
                        scalar1=lnwc[:, c:c + 1], scalar2=lnbc[:, c:c + 1],
                        op0=OP.mult, op1=OP.add)

        # ---------------- tab LN (1 row) + tnT ----------------
        tstats = ln_p.tile([1, 6], F32, name="tstats", tag="tstats")
        nc.vector.bn_stats(out=tstats, in_=tb)
        tmv = ln_p.tile([1, 2], F32, name="tmv", tag="tmv")
        nc.vector.bn_aggr(out=tmv, in_=tstats)
        tsd = ln_p.tile([1, 1], F32, name="tsd", tag="tsd")
        nc.scalar.activation(out=tsd, in_=tmv[:, 1:2], func=AF.Sqrt,
                             bias=eps_t[0:1], scale=1.0)
        trstd = ln_p.tile([1, 1], F32, name="trstd", tag="trstd")
        nc.vector.reciprocal(out=trstd, in_=tsd)
        tn = ln_p.tile([1, DIM], F32, name="tn", tag="tn", bufs=1)
        nc.vector.tensor_scalar(out=tn, in0=tb, scalar1=tmv[:, 0:1],
                                scalar2=trstd, op0=OP.subtract, op1=OP.mult)
        nc.vector.tensor_tensor(out=tn, in0=tn, in1=lnw_bc, op=OP.mult)
        tnb = ln_p.tile([1, DIM], BF16, name="tnb", tag="tnb", bufs=1)
        nc.vector.tensor_tensor(out=tnb, in0=tn, in1=lnb_bc, op=OP.add)
        for c in range(NC_):
            pt = psum_f.tile([128, 1], BF16, name="ptn", tag="fin")
            nc.tensor.transpose(out=pt, in_=tnb[0:1, c * 128:(c + 1) * 128],
                                identity=identb[0:1, 0:1])
            nc.vector.tensor_copy(out=tnT[:, c:c + 1], in_=pt)

        # zero-init staging so full-span strided ops never read uninitialized
        # rows (emitted after the LN loop: keeps the ACT/DVE front clear)
        nc.scalar.memzero(tabstr)
        nc.scalar.memzero(rcp8)
        nc.gpsimd.memset(dall, 1.0)
        if dbg is not None:
            nc.gpsimd.memset(rcp_s, 0.0)

        # ---------------- emit helpers ----------------
        def emit_tab_kv():
            for c in range(NC_):
                ps = psum_f.tile([128, 1], F32, name="pskt", tag="fin")
                for kc in range(NC_):
                    nc.tensor.matmul(
                        ps,
                        lhsT=wt[kc][:, c * 128:(c + 1) * 128],
                        rhs=tnT[:, kc:kc + 1],
                        start=(kc == 0), stop=(kc == NC_ - 1))
                nc.vector.tensor_copy(out=k_tT[:, c:c + 1], in_=ps)
            ps_vt = psum_f.tile([1, INNER], F32, name="psvt", tag="fin")
            for kc in range(NC_):
                nc.tensor.matmul(
                    ps_vt,
                    lhsT=tnT[:, kc:kc + 1],
                    rhs=wt[kc][:, INNER:2 * INNER],
                    start=(kc == 0), stop=(kc == NC_ - 1))
            vt_b = ln_p.tile([1, INNER], BF16, name="vt_b", tag="vt_b", bufs=1)
            nc.vector.tensor_copy(out=vt_b, in_=ps_vt)
            for c in range(NC_):
                pt = psum_f.tile([128, 1], BF16, name="ptv", tag="fin")
                nc.tensor.transpose(out=pt, in_=vt_b[0:1, c * 128:(c + 1) * 128],
                                    identity=identb[0:1, 0:1])
                nc.vector.tensor_copy(out=v_tT[:, c:c + 1], in_=pt)
            # scatter into zero-padded blocks: head h = 2c+j lives at
            # col (8c + h) rows 64j:64j+64 of kpad/vpad
            for c in range(NC_):
                for j in range(2):
                    h = 2 * c + j
                    nc.vector.tensor_copy(
                        out=kpad[64 * j:64 * j + 64, 8 * c + h:8 * c + h + 1],
                        in_=k_tT[64 * j:64 * j + 64, c:c + 1])
                    nc.vector.tensor_copy(
                        out=vpad[64 * j:64 * j + 64, 8 * c + h:8 * c + h + 1],
                        in_=v_tT[64 * j:64 * j + 64, c:c + 1])

        def emit_qkT(hp):
            # qT[hp] then kT[hp]; PSUM copies split DVE (q half 0) / ACT
            for m in (hp, hp + 4):
                dst = qT[m] if m < 4 else kT[m - 4]
                ps = big_tile("psqk")
                for qb in range(NQB):
                    half = ps[:, qb * QB:(qb + 1) * QB]
                    for kc in range(NC_):
                        nc.tensor.matmul(
                            half,
                            lhsT=wq[kc][:, m * 128:(m + 1) * 128],
                            rhs=xnT[kc][:, qb * QB:(qb + 1) * QB],
                            start=(kc == 0), stop=(kc == NC_ - 1))
                nc.vector.tensor_copy(out=dst[:, 0:QB], in_=ps[:, 0:QB])
                nc.scalar.copy(out=dst[:, QB:2 * QB], in_=ps[:, QB:2 * QB])

        ae_tiles = {}

        def emit_dots_kp(hp, qb, kp, ae0, ae1):
            ps0 = big_tile("psd0")
            ps1 = big_tile("psd1")
            for i, kt in enumerate((2 * kp, 2 * kp + 1)):
                for hh, ps in ((0, ps0), (1, ps1)):
                    hb = hh * 64
                    nc.tensor.matmul(
                        ps[:, i * QB:(i + 1) * QB],
                        lhsT=kT[hp][hb:hb + 64, kt * 128:(kt + 1) * 128],
                        rhs=qT[hp][hb:hb + 64, qb * QB:(qb + 1) * QB],
                        start=True, stop=True)
            nc.scalar.activation(
                out=ae0[:, 2 * kp * QB:(2 * kp + 2) * QB],
                in_=ps0, func=AF.Exp, scale=SCALE)
            nc.scalar.activation(
                out=ae1[:, 2 * kp * QB:(2 * kp + 2) * QB],
                in_=ps1, func=AF.Exp, scale=SCALE)

        def emit_dots(hp, qb):
            ae0 = ae_p.tile([128, 8 * QB], BF16, name="ae0", tag="ae0")
            ae1 = ae_p.tile([128, 8 * QB], BF16, name="ae1", tag="ae1")
            ae_tiles[(hp, qb)] = (ae0, ae1)
            for kp in range(4):
                emit_dots_kp(hp, qb, kp, ae0, ae1)

        def drain_po(hp, qb, hh, po):
            h = 2 * hp + hh
            qs = slice(qb * QB, (qb + 1) * QB)
            nc.vector.tensor_copy(
                out=outuT[hp][64 * hh:64 * hh + 64, qs], in_=po[0:64, :])
            r, gc = slot(h)
            nc.vector.tensor_copy(
                out=dall[r:r + 1, gc + qb * QB:gc + (qb + 1) * QB],
                in_=po[64:65, :])

        def emit_av_dots(hpA, qbA, hpB=None, qbB=None):
            # attn@v for (hpA, qbA), interleaved at kpos-pair granularity
            # with the dots of (hpB, qbB) so PE stalls on exp drains are
            # bridged by attn@v matmuls.
            ae0A, ae1A = ae_tiles.pop((hpA, qbA))
            po0 = psum_o.tile([128, QB], F32, name="po0", tag="o")
            po1 = psum_o.tile([128, QB], F32, name="po1", tag="o")
            if hpB is not None:
                aeB0 = ae_p.tile([128, 8 * QB], BF16, name="ae0", tag="ae0")
                aeB1 = ae_p.tile([128, 8 * QB], BF16, name="ae1", tag="ae1")
                ae_tiles[(hpB, qbB)] = (aeB0, aeB1)
            # attn@v accumulation chains must stay contiguous: interleaving
            # 64-row-mode dots matmuls mid-chain forces PE mode switches
            # that corrupt the in-flight PSUM accumulation on hardware.
            h0, h1 = 2 * hpA, 2 * hpA + 1
            for hh, po, h, ae in ((0, po0, h0, ae0A), (1, po1, h1, ae1A)):
                for kt in range(NT):
                    nc.tensor.matmul(
                        po[0:65, :],
                        lhsT=v_sb[kt][:, 65 * h:65 * h + 65],
                        rhs=ae[:, kt * QB:(kt + 1) * QB],
                        start=(kt == 0), stop=(kt == NT - 1))
                if hpB is not None:
                    emit_dots_kp(hpB, qbB, 2 * hh, aeB0, aeB1)
                    emit_dots_kp(hpB, qbB, 2 * hh + 1, aeB0, aeB1)
                drain_po(hpA, qbA, hh, po)

        def emit_norm_hp(qb, hp, pe_bc=False):
            # normalize heads 2hp, 2hp+1 for q block qb; their denominator
            # slots are rows {64j, 64j+32} (j = hp%2) of col block g = hp//2
            g, j = hp // 2, hp % 2
            qs = slice(qb * QB, (qb + 1) * QB)
            rows = slice(64 * j, 64 * j + 33)
            cs = slice(g * NTOK + qb * QB, g * NTOK + (qb + 1) * QB)
            nc.vector.tensor_tensor(out=dall[rows, cs], in0=dall[rows, cs],
                                    in1=tabstr[rows, cs], op=OP.add)
            # custom DVE ucode ops silently no-op at nonzero partition
            # offsets on HW: anchor the recip span at partition 0 (the j=1
            # call recomputes rows 0:33 of the same block, idempotently)
            rows0 = slice(0, 64 * j + 33)
            nc.vector.reciprocal_approx_fast(out=rcp_s[rows0, cs],
                                             in_=dall[rows0, cs])
            nc.vector.tensor_copy(out=rcpb_s[rows, cs], in_=rcp_s[rows, cs])
            # repack the two recip rows for na8 (tab K=8 final row)
            r0, gc0 = slot(2 * hp)
            r1, gc1 = slot(2 * hp + 1)
            qcs0 = slice(gc0 + qb * QB, gc0 + (qb + 1) * QB)
            qcs1 = slice(gc1 + qb * QB, gc1 + (qb + 1) * QB)
            nc.gpsimd.dma_start(out=rcp8[2 * hp:2 * hp + 1, qs],
                                in_=rcp_s[r0:r0 + 1, qcs0])
            nc.gpsimd.dma_start(out=rcp8[2 * hp + 1:2 * hp + 2, qs],
                                in_=rcp_s[r1:r1 + 1, qcs1])
            bc = bc_p.tile([128, QB], BF16, name="bc", tag=f"bc{hp % 2}")
            if pe_bc:
                # kernel tail: broadcast via PE ones-matmul (no DRAM bounce)
                bps = psum_o.tile([128, QB], F32, name="bps", tag="o")
                nc.tensor.matmul(
                    bps[0:64, :], lhsT=ones64[r0:r0 + 1, :],
                    rhs=rcpb_s[r0:r0 + 1, gc0 + qb * QB:gc0 + (qb + 1) * QB],
                    start=True, stop=True, tile_position=(r0, 0))
                nc.tensor.matmul(
                    bps[64:128, :], lhsT=ones64[r1:r1 + 1, :],
                    rhs=rcpb_s[r1:r1 + 1, gc1 + qb * QB:gc1 + (qb + 1) * QB],
                    start=True, stop=True, tile_position=(r1, 64))
                nc.vector.tensor_copy(out=bc, in_=bps)
            else:
                nc.gpsimd.dma_start(out=dscr[2 * hp:2 * hp + 1, qs],
                                    in_=rcpb_s[r0:r0 + 1, qcs0])
                nc.gpsimd.dma_start(out=dscr[2 * hp + 1:2 * hp + 2, qs],
                                    in_=rcpb_s[r1:r1 + 1, qcs1])
                # broadcast reads must go through the sync DGE: the gpsimd
                # DMA path mishandles stride-0 (to_broadcast) source APs
                nc.sync.dma_start(
                    out=bc[0:64, :],
                    in_=dscr[2 * hp:2 * hp + 1, qs].to_broadcast([64, QB]))
                nc.sync.dma_start(
                    out=bc[64:128, :],
                    in_=dscr[2 * hp + 1:2 * hp + 2, qs].to_broadcast([64, QB]))
            nc.vector.tensor_tensor(
                out=outuT[hp][:, qs], in0=outuT[hp][:, qs], in1=bc, op=OP.mult)
            if hp == 3:
                # all 8 recip rows for this qb are now fresh
                nc.vector.tensor_tensor(out=na8[:, qs], in0=tabexp8[:, qs],
                                        in1=rcp8[:, qs], op=OP.mult)

        def emit_final(t):
            pf = psum_f.tile([128, DIM], F32, name="pf", tag="fin")
            for c in range(NC_):
                nc.tensor.matmul(
                    pf,
                    lhsT=outuT[c][:, t * 128:(t + 1) * 128],
                    rhs=wo[c],
                    start=(c == 0), stop=False)
            nc.tensor.matmul(
                pf,
                lhsT=na8[0:8, t * 128:(t + 1) * 128],
                rhs=W_vt,
                start=False, stop=True)
            fo = fo_p.tile([128, DIM], F32, name="fo", tag="fo")
            nc.vector.tensor_tensor(out=fo, in0=pf, in1=bout_bc, op=OP.add)
            nc.sync.dma_start(out=out_d[t * 128:(t + 1) * 128, :], in_=fo)

        # ---------------- PE issue order ----------------
        emit_qkT(0)
        emit_dots(0, 0)
        emit_tab_kv()
        emit_qkT(1)
        emit_dots(1, 0)
        emit_qkT(2)
        emit_qkT(3)

        # tab dots: one packed M=8 matmul chain per q block + single exp
        for qb in range(NQB):
            ptab = psum_o.tile([8, QB], F32, name="ptab", tag="o")
            for c in range(NC_):
                nc.tensor.matmul(
                    ptab,
                    lhsT=kpad[:, 8 * c:8 * c + 8],
                    rhs=qT[c][:, qb * QB:(qb + 1) * QB],
                    start=(c == 0), stop=(c == NC_ - 1))
            nc.scalar.activation(
                out=tabexp8[:, qb * QB:(qb + 1) * QB],
                in_=ptab, func=AF.Exp, scale=SCALE)
        # repack exp(tab dots) into the strided staging layout (DMA moves
        # rows to arbitrary partitions; DVE cannot)
        for h in range(HEADS):
            r, gc = slot(h)
            nc.gpsimd.dma_start(
                out=tabstr[r:r + 1, gc:gc + NTOK],
                in_=tabexp8[h:h + 1, :])

        # W_vt = v_t @ w_out (K=8-packed blocks against wo chunks)
        psW = psum_f.tile([8, DIM], F32, name="psW", tag="fin")
        for c in range(NC_):
            nc.tensor.matmul(
                psW,
                lhsT=vpad[:, 8 * c:8 * c + 8],
                rhs=wo[c],
                start=(c == 0), stop=(c == NC_ - 1))
        nc.vector.tensor_copy(out=W_vt, in_=psW)

        # v token-major (+ ones interleave)
        for t in range(NT):
            ps = big_tile("psv")
            pv = ps[:, 0:QB]
            for kc in range(NC_):
                nc.tensor.matmul(
                    pv,
                    lhsT=xnT[kc][:, t * 128:(t + 1) * 128],
                    rhs=wq[kc][:, 2 * INNER:3 * INNER],
                    start=(kc == 0), stop=(kc == NC_ - 1))
            vdst = v_sb[t].rearrange("p (h s) -> p h s", s=65)
            nc.vector.tensor_copy(out=vdst[:, :, 0:64],
                                  in_=pv.rearrange("p (h d) -> p h d", d=64))
            nc.vector.tensor_copy(
                out=vdst[:, :, 64:65],
                in_=ones64.rearrange("p (h o) -> p h o", o=1)[:, 0:8, :])

        # attention steady state
        emit_av_dots(0, 0, 2, 0); emit_norm_hp(0, 0)
        emit_av_dots(1, 0, 3, 0); emit_norm_hp(0, 1)
        emit_av_dots(2, 0, 0, 1); emit_norm_hp(0, 2)
        emit_av_dots(3, 0, 1, 1); emit_norm_hp(0, 3)
        emit_av_dots(0, 1, 2, 1); emit_norm_hp(1, 0)
        emit_final(0); emit_final(1)
        emit_av_dots(1, 1, 3, 1); emit_norm_hp(1, 1)
        emit_final(2); emit_final(3)
        emit_av_dots(2, 1)
        emit_av_dots(3, 1)
        emit_norm_hp(1, 2)
        emit_norm_hp(1, 3)
        for t in range(4, 8):
            emit_final(t)

        if dbg is not None:
            nc.sync.dma_start(out=dbg["dbg_qT0"], in_=qT[0])
            nc.sync.dma_start(out=dbg["dbg_kT0"], in_=kT[0])
            nc.sync.dma_start(out=dbg["dbg_v0"], in_=v_sb[0])
            nc.sync.dma_start(out=dbg["dbg_dall"], in_=dall)
            nc.sync.dma_start(out=dbg["dbg_rcp"], in_=rcp_s)
            nc.sync.dma_start(out=dbg["dbg_na8"], in_=na8)
            nc.sync.dma_start(out=dbg["dbg_te8"], in_=tabexp8)
            nc.sync.dma_start(out=dbg["dbg_ouT0"], in_=outuT[0])
            nc.sync.dma_start(out=dbg["dbg_ouT3"], in_=outuT[3])


_CACHED_NC = None


def _to_bf16(a):
    import ml_dtypes
    return np.ascontiguousarray(np.asarray(a, dtype=np.float32)).astype(
        ml_dtypes.bfloat16)


def kernel(**inputs):
    global _CACHED_NC
    img = np.ascontiguousarray(np.asarray(inputs["img"], dtype=np.float32))
    tab = np.ascontiguousarray(np.asarray(inputs["tab"], dtype=np.float32))
    w_qkv = _to_bf16(inputs["w_qkv"])
    w_tab_qkv = np.asarray(inputs["w_tab_qkv"], dtype=np.float32)
    w_tab = _to_bf16(w_tab_qkv[:, INNER:3 * INNER])
    w_out = _to_bf16(inputs["w_out"])
    b_out = np.asarray(inputs["b_out"], dtype=np.float32).reshape(1, DIM)
    ln_w = np.asarray(inputs["ln_w"], dtype=np.float32).reshape(1, DIM)
    ln_b = np.asarray(inputs["ln_b"], dtype=np.float32).reshape(1, DIM)

    if _CACHED_NC is None:
        _CACHED_NC = build_program()
    nc = _CACHED_NC

    in_maps = []
    for b in range(N_CORES):
        in_maps.append({
            "img_s": np.ascontiguousarray(img[b]),
            "tab_s": np.ascontiguousarray(tab[b]),
            "wq_b": w_qkv,
            "wt_b": w_tab,
            "wo_b": w_out,
            "b_out": b_out,
            "ln_w": ln_w,
            "ln_b": ln_b,
        })

    res = bass_utils.run_bass_kernel_spmd(nc, in_maps, core_ids=list(range(N_CORES)))
    out = np.stack([res.results[c]["out_s"] for c in range(N_CORES)], axis=0)
    return out.astype(np.float32)


if __name__ == "__main__":
    d = np.load("/root/problem/ref_data.npz")
    ins = {k: d[k] for k in ("img", "tab", "w_qkv", "w_tab_qkv", "w_out",
                             "b_out", "ln_w", "ln_b")}
    actual = kernel(**ins)
    expected = d["expected"]
    err = np.abs(actual - expected).max()
    rel = err / np.abs(expected).max()
    print("absmax err:", err, "rel:", rel)
